# revision 1
# baseline (speedup 1.0000x reference)
"""GNN CSPLayer kernel for 8 Trainium2 NeuronCores (Bass/Tile).

Strategy (src-range sharding, all dense except the dst-side gather):
 - nodes split into 8 ranges of 6272 (=49*128); each core owns the edges whose
   src falls in its range (edge counts balance to ~1/8 each).
 - edges sorted by src; 128-src-node static blocks ("supertiles"); each block's
   edges live in padded 128-edge chunks.
 - z1 = eW1a@nf[src] + eW1b@nf[dst] + eW1q@attrs + eb1 computed feature-major
   in PSUM: Pa-side via (Pa_block as lhsT) x (one-hot src indicator) matmul,
   q-side via small matmul, dst-side via indirect-DMA row gathers from a
   precomputed Pb table (fp16) transposed into PSUM by transpose-matmuls.
 - edge MLP layer 2 per chunk (h1 chunk as stationary), silu on ACT.
 - scatter-mean: per-chunk one-hot indicator matmul accumulating [seg, feat]
   in PSUM; node index == C row index by construction (static blocks).
 - node MLP + residual fully dense; host concatenates core outputs.
"""
import numpy as np
import ml_dtypes

N = 50000
H = 128
E = 1000000
NCORES = 8
NPC = 6272            # nodes per core (49 * 128)
ST = NPC // 128       # 49 supertiles (blocks) per core
NFULL = NPC * NCORES  # 50176 padded node space
PB_ROWS = NFULL       # Pb table rows (padded)

FP8_ONE = 0x38        # 1.0 in float8_e4m3


def _host_pack(inputs):
    """Build all per-core arrays. Returns (in_maps, static) where static has
    the shape parameters the device program needs."""
    nf = np.asarray(inputs["node_features"], np.float32)          # [N, 128]
    frac = np.asarray(inputs["frac_coords"], np.float32)          # [N, 3]
    lat = np.asarray(inputs["lattices"], np.float32)              # [G, 6]
    ei = np.asarray(inputs["edge_index"]).astype(np.int64)        # [2, E]
    e2g = np.asarray(inputs["edge2graph"]).astype(np.int64)       # [E]
    lf = np.asarray(inputs["l_f_features"], np.float32)           # [E, 3]
    eW1 = np.asarray(inputs["eW1"], np.float32)                   # [268, 128]
    eb1 = np.asarray(inputs["eb1"], np.float32)
    eW2 = np.asarray(inputs["eW2"], np.float32)
    eb2 = np.asarray(inputs["eb2"], np.float32)
    nW1 = np.asarray(inputs["nW1"], np.float32)                   # [256, 128]
    nb1 = np.asarray(inputs["nb1"], np.float32)
    nW2 = np.asarray(inputs["nW2"], np.float32)
    nb2 = np.asarray(inputs["nb2"], np.float32)

    src = ei[0].astype(np.int64)
    dst = ei[1].astype(np.int64)
    ne = src.shape[0]

    # per-edge attribute vector [12] = [lat(6), frac_diff(3), l_f(3)]
    frac_diff = np.mod(frac[dst] - frac[src], 1.0)
    attrs_full = np.concatenate([lat[e2g], frac_diff, lf], axis=1)  # [E, 12]

    # degree (over src) and inv count
    deg = np.bincount(src, minlength=N).astype(np.float32)
    inv = 1.0 / np.maximum(deg, 1.0)
    inv_pad = np.zeros(NFULL, np.float32)
    inv_pad[:N] = inv

    # padded transposed node features fp16
    nf_pad = np.zeros((NFULL, H), np.float32)
    nf_pad[:N] = nf
    nf16t_full = nf_pad.T.astype(np.float16).copy()               # [128, NFULL]

    # ---- per-core edge partitioning -------------------------------------
    core_of = (src // NPC).astype(np.int64)
    order = np.lexsort((src,))  # sorted by src globally
    src_s = src[order]
    core_s = core_of[order]
    core_starts = np.searchsorted(core_s, np.arange(NCORES + 1) * 0 + 0)  # placeholder

    per_core = []
    cnt_mat = np.zeros((NCORES, ST), np.int64)
    for c in range(NCORES):
        lo = np.searchsorted(src_s, c * NPC)
        hi = np.searchsorted(src_s, (c + 1) * NPC)
        eids = order[lo:hi]                    # edges sorted by src
        slocal = src[eids] - c * NPC
        blk = slocal >> 7
        cnt = np.bincount(blk, minlength=ST)
        cnt_mat[c] = cnt
        per_core.append((eids, slocal, blk, cnt))

    maxcnt = cnt_mat.max(axis=0)                                   # [ST]
    nch = np.maximum(1, (maxcnt + 127) // 128).astype(np.int64)    # chunks per st
    colbase = np.concatenate([[0], np.cumsum(nch)])                # [ST+1]
    totch = int(colbase[-1])
    slots_tot = totch * 128

    # ---- weights (fp16) --------------------------------------------------
    w_a16 = eW1[0:128].astype(np.float16)            # [128,128] hi part
    w_b16 = eW1[128:256].astype(np.float16)          # [128,128] hj part
    w_q16 = np.zeros((16, 128), np.float16)
    w_q16[0:12] = eW1[256:268].astype(np.float16)
    eW2_16 = eW2.astype(np.float16)
    nW1a16 = nW1[0:128].astype(np.float16)
    nW1b16 = nW1[128:256].astype(np.float16)
    nW2_16 = nW2.astype(np.float16)
    ident32 = np.eye(128, dtype=np.float32)
    ident16 = np.eye(128, dtype=np.float16)

    assert np.abs(eb2).max() == 0.0, "nonzero eb2 not folded (add matmul path)"

    in_maps = []
    for c in range(NCORES):
        eids, slocal, blk, cnt = per_core[c]
        nloc = slocal.shape[0]
        # position within block
        blk_starts = np.concatenate([[0], np.cumsum(cnt)])
        pos_in_blk = np.arange(nloc) - blk_starts[blk]
        slot = colbase[blk] * 128 + pos_in_blk                     # global slot id
        p = (pos_in_blk & 127).astype(np.int64)
        col = colbase[blk] + (pos_in_blk >> 7)

        dst_idx = np.zeros((128, totch), np.int32)
        dst_idx[p, col] = dst[eids].astype(np.int32)

        onehotA = np.zeros((128, slots_tot), np.uint8)             # [node_in_blk, slot]
        onehotA[slocal & 127, slot] = FP8_ONE

        onehotB = np.zeros((128, totch, 128), np.uint8)            # [p, col, seg]
        onehotB[p, col, slocal & 127] = FP8_ONE

        attrs16 = np.zeros((16, slots_tot), np.float16)
        attrs16[0:12, slot] = attrs_full[eids].T.astype(np.float16)

        g0 = c * NPC
        in_maps.append({
            "nf16t_full": nf16t_full,
            "nf16t": nf16t_full[:, g0:g0 + NPC].copy(),
            "nf_res": nf_pad[g0:g0 + NPC].copy(),
            "inv_d": inv_pad[g0:g0 + NPC].reshape(NPC, 1).copy(),
            "dst_idx": dst_idx,
            "onehotA": onehotA.view(ml_dtypes.float8_e4m3),
            "onehotB": onehotB.reshape(128, totch * 128).view(ml_dtypes.float8_e4m3),
            "attrs16": attrs16,
            "w_a16": w_a16, "w_b16": w_b16, "w_q16": w_q16,
            "eW2_16": eW2_16, "nW1a16": nW1a16, "nW1b16": nW1b16,
            "nW2_16": nW2_16,
            "eb1": eb1.reshape(128, 1).astype(np.float32),
            "nb1": nb1.reshape(128, 1).astype(np.float32),
            "nb2": nb2.reshape(128, 1).astype(np.float32),
            "ident32": ident32, "ident16": ident16,
        })

    static = {"nch": [int(x) for x in nch], "totch": totch,
              "slots_tot": slots_tot, "colbase": [int(x) for x in colbase]}
    return in_maps, static


def _build_nc(static):
    import concourse.bass as bass
    import concourse.bacc as bacc
    import concourse.tile as tile
    import concourse.mybir as mybir

    dt = mybir.dt
    nch = static["nch"]
    totch = static["totch"]
    slots_tot = static["slots_tot"]
    colbase = static["colbase"]

    nc = bacc.Bacc("TRN2", target_bir_lowering=False, debug=False,
                   num_devices=NCORES)

    def di(name, shape, dtype):
        return nc.dram_tensor(name, shape, dtype, kind="ExternalInput").ap()

    nf16t_full = di("nf16t_full", [128, NFULL], dt.float16)
    nf16t = di("nf16t", [128, NPC], dt.float16)
    nf_res = di("nf_res", [NPC, 128], dt.float32)
    inv_d = di("inv_d", [NPC, 1], dt.float32)
    dst_idx = di("dst_idx", [128, totch], dt.int32)
    onehotA = di("onehotA", [128, slots_tot], dt.float8e4)
    onehotB = di("onehotB", [128, totch * 128], dt.float8e4)
    attrs16 = di("attrs16", [16, slots_tot], dt.float16)
    w_a16 = di("w_a16", [128, 128], dt.float16)
    w_b16 = di("w_b16", [128, 128], dt.float16)
    w_q16 = di("w_q16", [16, 128], dt.float16)
    eW2_16 = di("eW2_16", [128, 128], dt.float16)
    nW1a16 = di("nW1a16", [128, 128], dt.float16)
    nW1b16 = di("nW1b16", [128, 128], dt.float16)
    nW2_16 = di("nW2_16", [128, 128], dt.float16)
    eb1 = di("eb1", [128, 1], dt.float32)
    nb1 = di("nb1", [128, 1], dt.float32)
    nb2 = di("nb2", [128, 1], dt.float32)
    ident32 = di("ident32", [128, 128], dt.float32)
    ident16 = di("ident16", [128, 128], dt.float16)

    # scratch DRAM
    pa_dram = nc.dram_tensor("pa_dram", [NPC, 128], dt.float16, kind="Internal").ap()
    pb_dram = nc.dram_tensor("pb_dram", [PB_ROWS, 128], dt.float16, kind="Internal").ap()
    c_dram = nc.dram_tensor("c_dram", [NPC, 128], dt.float32, kind="Internal").ap()
    out_d = nc.dram_tensor("out", [NPC, 128], dt.float32, kind="ExternalOutput").ap()

    AF = mybir.ActivationFunctionType

    with tile.TileContext(nc) as tc:
        with (
            tc.tile_pool(name="const", bufs=1) as pconst,
            tc.tile_pool(name="stream", bufs=3) as pstream,
            tc.tile_pool(name="gath", bufs=16) as pgath,
            tc.tile_pool(name="work", bufs=3) as pwork,
            tc.tile_pool(name="pz1", bufs=2, space="PSUM") as pz1,
            tc.tile_pool(name="pT", bufs=2, space="PSUM") as pT,
            tc.tile_pool(name="pef", bufs=2, space="PSUM") as pef,
            tc.tile_pool(name="pC", bufs=2, space="PSUM") as pC,
        ):
            # ---- constants to SBUF ----
            c_wa = pconst.tile([128, 128], dt.float16)
            c_wb = pconst.tile([128, 128], dt.float16)
            c_wq = pconst.tile([16, 128], dt.float16)
            c_ew2 = pconst.tile([128, 128], dt.float16)
            c_nw1a = pconst.tile([128, 128], dt.float16)
            c_nw1b = pconst.tile([128, 128], dt.float16)
            c_nw2 = pconst.tile([128, 128], dt.float16)
            c_eb1 = pconst.tile([128, 1], dt.float32)
            c_nb1 = pconst.tile([128, 1], dt.float32)
            c_nb2 = pconst.tile([128, 1], dt.float32)
            c_id32 = pconst.tile([128, 128], dt.float32)
            c_id16 = pconst.tile([128, 128], dt.float16)
            c_idx = pconst.tile([128, totch], dt.int32)
            for t, d in [(c_wa, w_a16), (c_wb, w_b16), (c_wq, w_q16),
                         (c_ew2, eW2_16), (c_nw1a, nW1a16), (c_nw1b, nW1b16),
                         (c_nw2, nW2_16), (c_eb1, eb1), (c_nb1, nb1),
                         (c_nb2, nb2), (c_id32, ident32), (c_id16, ident16),
                         (c_idx, dst_idx)]:
                nc.sync.dma_start(t[:], d[:])

            # ---- preamble: Pa / Pb tables (batched by 4 chunks) ---------
            def build_table(src_fm, ncols, w_tile, table):
                nchk = ncols // 128
                j = 0
                while j < nchk:
                    g = min(4, nchk - j)
                    w = g * 128
                    nft = pstream.tile([128, 512], dt.float16, tag="nft")
                    nc.sync.dma_start(nft[:, 0:w], src_fm[:, j * 128:j * 128 + w])
                    ps = pT.tile([128, 4, 128], dt.float32, tag="t")
                    for k in range(g):
                        nc.tensor.matmul(ps[:, k, :],
                                         nft[:, k * 128:(k + 1) * 128],
                                         w_tile[:], start=True, stop=True)
                    sb = pstream.tile([128, 4, 128], dt.float16, tag="pre_sb")
                    nc.vector.tensor_copy(
                        sb[:, 0:g, :].rearrange("p a b -> p (a b)"),
                        ps[:, 0:g, :].rearrange("p a b -> p (a b)"))
                    nc.sync.dma_start(
                        table[j * 128:j * 128 + w, :].rearrange(
                            "(a p) b -> p a b", p=128),
                        sb[:, 0:g, :])
                    j += g

            build_table(nf16t_full, NFULL, c_wb, pb_dram)
            build_table(nf16t, NPC, c_wa, pa_dram)

            # ---- edge phase ---------------------------------------------
            for st in range(ST):
                k0 = colbase[st]
                nk = nch[st]
                nslots = nk * 128

                pa_blk = pstream.tile([128, 128], dt.float16, tag="pa_blk")
                nc.sync.dma_start(pa_blk[:], pa_dram[st * 128:(st + 1) * 128, :])
                ohA = pstream.tile([128, nslots], dt.float8e4, tag="ohA")
                nc.sync.dma_start(ohA[:], onehotA[:, k0 * 128:k0 * 128 + nslots])
                ohB = pstream.tile([128, nslots], dt.float8e4, tag="ohB")
                nc.sync.dma_start(ohB[:], onehotB[:, k0 * 128:k0 * 128 + nslots])
                att = pstream.tile([16, nslots], dt.float16, tag="att")
                nc.sync.dma_start(att[:], attrs16[:, k0 * 128:k0 * 128 + nslots])

                psC = pC.tile([128, 128], dt.float32, tag="C")
                first_seg_mm = [True]

                # gathers for all chunks of this supertile
                zbs = []
                for k in range(nk):
                    zb = pgath.tile([128, 128], dt.float16, tag="zb")
                    nc.gpsimd.indirect_dma_start(
                        out=zb[:], out_offset=None, in_=pb_dram[:, :],
                        in_offset=bass.IndirectOffsetOnAxis(
                            ap=c_idx[:, k0 + k:k0 + k + 1], axis=0),
                    )
                    zbs.append(zb)

                ngrp = (nk + 3) // 4
                for g in range(ngrp):
                    kg0 = g * 4
                    kgn = min(4, nk - kg0)
                    w = kgn * 128
                    s0 = kg0 * 128

                    # feature-major z1 partial: q + Pa parts
                    ps_z1 = pz1.tile([128, 512], dt.float32, tag="ps_z1")
                    nc.tensor.matmul(ps_z1[:, 0:w], c_wq[:], att[:, s0:s0 + w],
                                     start=True, stop=False)
                    nc.tensor.matmul(ps_z1[:, 0:w], pa_blk[:],
                                     ohA[:, s0:s0 + w],
                                     start=False, stop=True, skip_group_check=True)

                    # transpose gathered dst rows into feature-major psum
                    ps_t = pT.tile([128, 512], dt.float16, tag="t")
                    for kk in range(kgn):
                        nc.tensor.matmul(
                            ps_t[:, kk * 128:(kk + 1) * 128],
                            zbs[kg0 + kk][:], c_id16[:],
                            is_transpose=True, start=True, stop=True,
                        )
                    zb_fm = pwork.tile([128, 512], dt.float16, tag="zb_fm")
                    nc.vector.tensor_copy(zb_fm[:, 0:w], ps_t[:, 0:w])
                    z1sb = pwork.tile([128, 512], dt.float16, tag="z1sb")
                    nc.vector.tensor_add(z1sb[:, 0:w], zb_fm[:, 0:w], ps_z1[:, 0:w])

                    h1 = pwork.tile([128, 512], dt.float16, tag="h1")
                    nc.scalar.activation(h1[:, 0:w], z1sb[:, 0:w], AF.Silu,
                                         bias=c_eb1[:])

                    # layer 2 per chunk: ef edge-major
                    ps_ef = pef.tile([128, 4, 128], dt.float32, tag="ef_ps")
                    for kk in range(kgn):
                        nc.tensor.matmul(
                            ps_ef[:, kk, :],
                            h1[:, kk * 128:(kk + 1) * 128], c_ew2[:],
                            start=True, stop=True,
                        )
                    ef = pwork.tile([128, 4, 128], dt.float16, tag="ef")
                    nc.scalar.activation(
                        ef[:, 0:kgn, :].rearrange("p a b -> p (a b)"),
                        ps_ef[:, 0:kgn, :].rearrange("p a b -> p (a b)"),
                        AF.Silu)

                    # segment sum: onehotB chunk as stationary, ef as moving
                    for kk in range(kgn):
                        k = kg0 + kk
                        nc.tensor.matmul(
                            psC[:], ohB[:, k * 128:(k + 1) * 128],
                            ef[:, kk, :],
                            start=first_seg_mm[0], stop=(k == nk - 1),
                            skip_group_check=True,
                        )
                        first_seg_mm[0] = False

                csb = pwork.tile([128, 128], dt.float32, tag="csb")
                nc.vector.tensor_copy(csb[:], psC[:])
                nc.sync.dma_start(c_dram[st * 128:(st + 1) * 128, :], csb[:])

            # ---- node phase ---------------------------------------------
            tiles = [(i * 512, 512) for i in range(12)] + [(12 * 512, 128)]
            for (t0, tw) in tiles:
                nchk = tw // 128
                agg_fm = pwork.tile([128, 512], dt.float16, tag="agg_fm")
                crow = pstream.tile([128, 4, 128], dt.float32, tag="crow")
                nc.sync.dma_start(
                    crow[:, 0:nchk, :],
                    c_dram[t0:t0 + tw, :].rearrange("(a p) b -> p a b", p=128))
                invt = pstream.tile([128, 4, 1], dt.float32, tag="invt")
                nc.sync.dma_start(
                    invt[:, 0:nchk, :],
                    inv_d[t0:t0 + tw, :].rearrange("(a p) o -> p a o", p=128))
                for j in range(nchk):
                    aggs = pwork.tile([128, 128], dt.float32, tag="aggs")
                    nc.vector.tensor_scalar_mul(aggs[:], crow[:, j, :],
                                                invt[:, j, :])
                    ps_at = pT.tile([128, 128], dt.float32, tag="t")
                    nc.tensor.matmul(ps_at[:], aggs[:], c_id32[:],
                                     is_transpose=True, start=True, stop=True)
                    nc.vector.tensor_copy(agg_fm[:, j * 128:(j + 1) * 128], ps_at[:])

                nft = pstream.tile([128, 512], dt.float16, tag="nft2")
                nc.sync.dma_start(nft[:, 0:tw], nf16t[:, t0:t0 + tw])
                ps_z = pz1.tile([128, 512], dt.float32, tag="ps_z1")
                nc.tensor.matmul(ps_z[:, 0:tw], c_nw1a[:], nft[:, 0:tw],
                                 start=True, stop=False)
                nc.tensor.matmul(ps_z[:, 0:tw], c_nw1b[:], agg_fm[:, 0:tw],
                                 start=False, stop=True, skip_group_check=True)
                gt = pwork.tile([128, 512], dt.float16, tag="gt")
                nc.scalar.activation(gt[:, 0:tw], ps_z[:, 0:tw], AF.Silu,
                                     bias=c_nb1[:])
                ps_z2 = pef.tile([128, 512], dt.float32, tag="ef_ps")
                nc.tensor.matmul(ps_z2[:, 0:tw], c_nw2[:], gt[:, 0:tw],
                                 start=True, stop=True)
                s2 = pwork.tile([128, 512], dt.float32, tag="s2")
                nc.scalar.activation(s2[:, 0:tw], ps_z2[:, 0:tw], AF.Silu,
                                     bias=c_nb2[:])
                res = pstream.tile([128, 4, 128], dt.float32, tag="res")
                nc.sync.dma_start(
                    res[:, 0:nchk, :],
                    nf_res[t0:t0 + tw, :].rearrange("(a p) b -> p a b", p=128))
                osb = pwork.tile([128, 4, 128], dt.float32, tag="osb")
                for j in range(nchk):
                    ps_o = pC.tile([128, 128], dt.float32, tag="C")
                    nc.tensor.matmul(ps_o[:], s2[:, j * 128:(j + 1) * 128],
                                     c_id32[:], is_transpose=True,
                                     start=True, stop=True)
                    nc.vector.tensor_add(osb[:, j, :], res[:, j, :], ps_o[:])
                nc.sync.dma_start(
                    out_d[t0:t0 + tw, :].rearrange("(a p) b -> p a b", p=128),
                    osb[:, 0:nchk, :])

    nc.compile()
    return nc


def kernel(**inputs):
    from concourse.bass_utils import run_bass_kernel_spmd

    in_maps, static = _host_pack(inputs)
    nc = _build_nc(static)
    res = run_bass_kernel_spmd(nc, in_maps, core_ids=list(range(NCORES)))
    outs = [res.results[c]["out"] for c in range(NCORES)]
    full = np.concatenate(outs, axis=0)[:N]
    return full.astype(np.float32)

